# revision 13
# baseline (speedup 1.0000x reference)
"""Trainium2 Bass kernel for nn_ConditionedDense (hypernetwork-conditioned dense).

Reference computation:
    A = einsum('bnp,pq->bnq', P, Wk)         # hypernetwork: per-position weights
    W = relu(A).reshape(B, N, c_in, c_out)
    out = einsum('bni,bnio->bno', X, W)

Strategy: pure data parallel over 8 NeuronCores (shard batch dim), 16384
positions per core, processed in 16 chunks of 1024 positions. Two pipelines
share the chunks so that PE, ACT and DVE all stay busy:

TREE chunks ([pos, q] layout):
  - PE matmul per 128-pos tile: A = P_tile @ Wk into PSUM
  - ACT relu evicts PSUM -> SBUF bf16
  - DVE multiplies by X (broadcast over o, 2x bf16) and reduces over i with
    a 5-level halving tree of tensor_tensor adds

AT chunks (A^T [q, pos] layout, 2 blocks of 512 positions each):
  - PE matmul per q-chunk g: A^T[q128, pos512] = Wk[:,g]^T-stationary @ P^T
  - eviction+relu+multiply: either fused DVE scalar_tensor_tensor
    ((A max 0) * X_rep, 1x from PSUM) or ACT relu + DVE multiply (2x)
  - the i-reduction runs on the PE: 8 accumulating matmuls with constant
    selection matrices S_g produce out^T[32, 512] in PSUM
  - ACT evicts out^T, host re-transposes

HAM clock-gate management: the PE only un-throttles (1.2 -> 2.4 GHz) under
K=128 matmul activity; K=64 matmuls never warm it and let it re-throttle.
All matmuls here are therefore K=128: P^T lives in persistent [128, ...]
buffers whose bottom 64 partitions are zeroed once (zeros make the padding
exact), and a 12-matmul back-to-back primer warms the PE at kernel start.

Host side (free): P transposed, X transposed layout, Wk column permuted to
q = o*32+i, selection matrices, bf16 casts, output merge.
"""

import os
from contextlib import ExitStack

import numpy as np
import ml_dtypes

import concourse.bass as bass
import concourse.tile as tile
from concourse import bacc, mybir
from concourse.bass_utils import run_bass_kernel_spmd

C_IN = 32
C_OUT = 32
P_DIM = 64
Q = C_IN * C_OUT  # 1024
B, N = 32, 4096
N_CORES = 8
B_SH = B // N_CORES          # 4 batches per core
NPOS = B_SH * N              # 16384 positions per core
TILE_P = 128                 # positions per tile
N_TILES = NPOS // TILE_P     # 128
CHUNK = 8                    # tiles per chunk
N_CHUNKS = N_TILES // CHUNK  # 16
BLK = 512                    # AT-block positions
NG = Q // 128                # 8 q-chunks of 128

N_PRIMER = 12                # redundant K=128 warm-up matmuls
TREE_CHUNKS = (0, 1, 2, 3, 4, 6, 8, 10, 12, 13, 14)
# AT blocks evicted via fused DVE scalar_tensor_tensor: (chunk, block) pairs
STT_BLOCKS = frozenset({(5, 1), (7, 1), (9, 1), (11, 1), (15, 1)})
# tree tiles evicted via fused DVE scalar_tensor_tensor: (chunk, tile) pairs
STT_TREE = frozenset({(1, 7), (2, 7), (3, 7), (4, 7)})

F32 = mybir.dt.float32
BF16 = mybir.dt.bfloat16

_BUILD_CACHE = {}
LAST_RESULTS = None  # BassKernelResults of the most recent run (for profiling)


def _build_nc():
    nc = bacc.Bacc(
        "TRN2", target_bir_lowering=False, debug=False, num_devices=N_CORES
    )
    X_d = nc.declare_dram_parameter("X", [NPOS, C_IN], BF16, isOutput=False)
    XT_d = nc.declare_dram_parameter("XT", [C_IN, NPOS], BF16, isOutput=False)
    PT_d = nc.declare_dram_parameter("PT", [P_DIM, NPOS], BF16, isOutput=False)
    Wk_d = nc.declare_dram_parameter("Wk", [P_DIM, Q], BF16, isOutput=False)
    S_d = nc.declare_dram_parameter("S", [128, NG * C_OUT], BF16, isOutput=False)
    out_d = nc.declare_dram_parameter("out", [NPOS, C_OUT], BF16, isOutput=True)
    outT_d = nc.declare_dram_parameter("outT", [C_OUT, NPOS], BF16, isOutput=True)

    relu = mybir.ActivationFunctionType.Relu
    mult = mybir.AluOpType.mult
    add = mybir.AluOpType.add
    amax = mybir.AluOpType.max

    tree_set = set(TREE_CHUNKS)

    with ExitStack() as ctx:
        tc = ctx.enter_context(tile.TileContext(nc))
        wkp = ctx.enter_context(tc.tile_pool(name="wk", bufs=1))
        xp = ctx.enter_context(tc.tile_pool(name="x", bufs=2))
        xrp = ctx.enter_context(tc.tile_pool(name="xr", bufs=2))
        pp = ctx.enter_context(tc.tile_pool(name="pT", bufs=1))
        # PSUM: tree pool + AT pool + out2 pool
        tap = ctx.enter_context(tc.tile_pool(name="tpsum", bufs=2, space="PSUM"))
        aap = ctx.enter_context(tc.tile_pool(name="apsum", bufs=3, space="PSUM"))
        o2p = ctx.enter_context(tc.tile_pool(name="o2psum", bufs=1, space="PSUM"))
        hp = ctx.enter_context(tc.tile_pool(name="h", bufs=2))
        hatp = ctx.enter_context(tc.tile_pool(name="hat", bufs=2))
        mp = ctx.enter_context(tc.tile_pool(name="m", bufs=2))
        matp = ctx.enter_context(tc.tile_pool(name="mat", bufs=2))
        t1p = ctx.enter_context(tc.tile_pool(name="t1", bufs=2))
        t2p = ctx.enter_context(tc.tile_pool(name="t2", bufs=2))
        t3p = ctx.enter_context(tc.tile_pool(name="t3", bufs=2))
        t4p = ctx.enter_context(tc.tile_pool(name="t4", bufs=2))
        op = ctx.enter_context(tc.tile_pool(name="o", bufs=3))
        o2sp = ctx.enter_context(tc.tile_pool(name="o2s", bufs=3))

        # K=128 primer operands first in the GPSIMD queue so the warm-up
        # matmuls (emitted in chunk 0) can issue while parameter DMAs run.
        prime_l = wkp.tile([128, 128], BF16)
        prime_r = wkp.tile([128, 512], BF16)
        nc.vector.memset(prime_l[:], 0)
        nc.vector.memset(prime_r[:], 0)

        # Wk padded to K=128: rows 64..127 only ever multiply the zeroed
        # bottom halves of the P^T buffers, so their content is irrelevant;
        # zero them anyway.
        wk_t = wkp.tile([128, Q], BF16)
        nc.gpsimd.memset(wk_t[:], 0)
        nc.sync.dma_start(out=wk_t[0:P_DIM, :], in_=Wk_d[:])
        s_t = wkp.tile([128, NG * C_OUT], BF16)
        nc.sync.dma_start(out=s_t[:], in_=S_d[:])

        # persistent P^T ring: [128, 1024] buffers, bottom 64 rows zeroed
        # once so every matmul can run with K=128 (keeps the HAM warm)
        N_PT = 3
        pT_bufs = [
            pp.tile([128, CHUNK * TILE_P], BF16, name=f"pT{i}")
            for i in range(N_PT)
        ]
        for t in pT_bufs:
            nc.gpsimd.memset(t[P_DIM:128, :], 0)

        # deferred emission of an AT block's reduce matmuls + output, so the
        # PE has the next block's gen matmuls queued before a data wait
        pending = []

        def flush_pending():
            while pending:
                fn = pending.pop(0)
                fn()

        def emit_at_reduce(ch, b, m_at):
            def go():
                o2_t = o2p.tile([C_OUT, BLK], F32)
                for g in range(NG):
                    nc.tensor.matmul(
                        o2_t[:], lhsT=s_t[:, bass.ts(g, C_OUT)],
                        rhs=m_at[:, g, :],
                        start=(g == 0), stop=(g == NG - 1),
                    )
                o2s_t = o2sp.tile([C_OUT, BLK], BF16)
                nc.scalar.activation(
                    o2s_t[:], o2_t[:], mybir.ActivationFunctionType.Copy
                )
                nc.sync.dma_start(
                    out=outT_d[:, ch * CHUNK * TILE_P + b * BLK:
                               ch * CHUNK * TILE_P + (b + 1) * BLK],
                    in_=o2s_t[:],
                )
            pending.append(go)

        for ch in range(N_CHUNKS):
            pT_c = pT_bufs[ch % N_PT]
            nc.sync.dma_start(
                out=pT_c[0:P_DIM, :], in_=PT_d[:, bass.ts(ch, TILE_P * CHUNK)]
            )

            if ch in tree_set:
                # ---------------- TREE chunk ----------------
                x_c = xp.tile([TILE_P, CHUNK, C_IN], BF16)
                nc.sync.dma_start(
                    out=x_c[:],
                    in_=X_d[bass.ts(ch, TILE_P * CHUNK), :].rearrange(
                        "(a p) i -> p a i", p=TILE_P
                    ),
                )
                h_c = hp.tile([TILE_P, CHUNK, Q], BF16)
                m_c = mp.tile([TILE_P, CHUNK, Q], BF16)
                o_c = op.tile([TILE_P, CHUNK, C_OUT], BF16)

                for t in range(CHUNK):
                    a_t = tap.tile([TILE_P, Q], F32)
                    if ch == 0 and t == 0:
                        for w in range(N_PRIMER):
                            nc.tensor.matmul(
                                a_t[:, 0:512], lhsT=prime_l[:],
                                rhs=prime_r[:], start=True, stop=True,
                            )
                    lhsT = pT_c[:, bass.ts(t, TILE_P)]
                    nc.tensor.matmul(
                        a_t[:, 0:512], lhsT=lhsT, rhs=wk_t[:, 0:512],
                        start=True, stop=True,
                    )
                    nc.tensor.matmul(
                        a_t[:, 512:1024], lhsT=lhsT, rhs=wk_t[:, 512:1024],
                        start=True, stop=True,
                    )
                    if (ch, t) in STT_TREE:
                        nc.vector.scalar_tensor_tensor(
                            out=m_c[:, t, :].rearrange(
                                "p (o i) -> p o i", o=C_OUT
                            ),
                            in0=a_t[:].rearrange("p (o i) -> p o i", o=C_OUT),
                            scalar=0.0,
                            in1=x_c[:, t, :].unsqueeze(1).broadcast_to(
                                [TILE_P, C_OUT, C_IN]
                            ),
                            op0=amax, op1=mult,
                        )
                    else:
                        nc.scalar.activation(h_c[:, t, :], a_t[:], relu)
                    if t == 1:
                        flush_pending()

                n_act = CHUNK - (1 if any(
                    (ch, t) in STT_TREE for t in range(CHUNK)
                ) else 0)
                h4 = h_c[:, 0:n_act, :].rearrange(
                    "p j (o i) -> p j o i", o=C_OUT
                )
                m4 = m_c[:, 0:n_act, :].rearrange(
                    "p j (o i) -> p j o i", o=C_OUT
                )
                x4 = x_c[:, 0:n_act, :].unsqueeze(2).broadcast_to(
                    [TILE_P, n_act, C_OUT, C_IN]
                )
                nc.vector.tensor_tensor(out=m4, in0=h4, in1=x4, op=mult)
                m4 = m_c[:].rearrange("p j (o i) -> p j o i", o=C_OUT)

                t1 = t1p.tile([TILE_P, CHUNK, C_OUT, 16], BF16)
                nc.vector.tensor_tensor(
                    out=t1[:], in0=m4[:, :, :, 0:16], in1=m4[:, :, :, 16:32],
                    op=add,
                )
                t2 = t2p.tile([TILE_P, CHUNK, C_OUT, 8], BF16)
                nc.vector.tensor_tensor(
                    out=t2[:], in0=t1[:, :, :, 0:8], in1=t1[:, :, :, 8:16],
                    op=add,
                )
                t3 = t3p.tile([TILE_P, CHUNK, C_OUT, 4], BF16)
                nc.vector.tensor_tensor(
                    out=t3[:], in0=t2[:, :, :, 0:4], in1=t2[:, :, :, 4:8],
                    op=add,
                )
                t4 = t4p.tile([TILE_P, CHUNK, C_OUT, 2], BF16)
                nc.vector.tensor_tensor(
                    out=t4[:], in0=t3[:, :, :, 0:2], in1=t3[:, :, :, 2:4],
                    op=add,
                )
                nc.vector.tensor_tensor(
                    out=o_c[:].unsqueeze(3), in0=t4[:, :, :, 0:1],
                    in1=t4[:, :, :, 1:2], op=add,
                )
                nc.sync.dma_start(
                    out=out_d[bass.ts(ch, TILE_P * CHUNK), :].rearrange(
                        "(a p) i -> p a i", p=TILE_P
                    ),
                    in_=o_c[:],
                )
            else:
                # ---------------- AT chunk (2 blocks of 512 pos) ----------
                xr_c = xrp.tile([128, 2, BLK], BF16)
                for r in range(4):
                    nc.sync.dma_start(
                        out=xr_c[32 * r:32 * (r + 1), :, :],
                        in_=XT_d[:, bass.ts(ch, TILE_P * CHUNK)].rearrange(
                            "i (b c) -> i b c", b=2
                        ),
                    )
                for b in range(2):
                    is_stt = (ch, b) in STT_BLOCKS
                    m_at = matp.tile([128, NG, BLK], BF16)
                    h_at = None if is_stt else hatp.tile([128, NG, BLK], BF16)
                    for g in range(NG):
                        a_at = aap.tile([128, BLK], F32)
                        nc.tensor.matmul(
                            a_at[:], lhsT=wk_t[:, bass.ts(g, 128)],
                            rhs=pT_c[:, bass.ts(b, BLK)],
                            start=True, stop=True,
                        )
                        if is_stt:
                            nc.vector.scalar_tensor_tensor(
                                out=m_at[:, g, :], in0=a_at[:], scalar=0.0,
                                in1=xr_c[:, b, :], op0=amax, op1=mult,
                            )
                        else:
                            nc.scalar.activation(h_at[:, g, :], a_at[:], relu)
                        if g == 1:
                            flush_pending()
                    if not is_stt:
                        nc.vector.tensor_tensor(
                            out=m_at[:], in0=h_at[:],
                            in1=xr_c[:, b, :].unsqueeze(1).broadcast_to(
                                [128, NG, BLK]
                            ),
                            op=mult,
                        )
                    emit_at_reduce(ch, b, m_at)
        flush_pending()

    nc.finalize()
    return nc


def _get_nc():
    key = "v4"
    if key not in _BUILD_CACHE:
        _BUILD_CACHE[key] = _build_nc()
    return _BUILD_CACHE[key]


def kernel(X, P, Wk):
    global LAST_RESULTS
    X = np.asarray(X, dtype=np.float32)
    P = np.asarray(P, dtype=np.float32)
    Wk = np.asarray(Wk, dtype=np.float32)
    bf16 = ml_dtypes.bfloat16

    # Host-side prep (free): shard, transpose P/X, permute Wk columns so the
    # device-side layout is q = o*32 + i; selection matrices for the PE
    # reduction; cast matmul operands to bf16.
    WkP = np.ascontiguousarray(
        Wk.reshape(P_DIM, C_IN, C_OUT).transpose(0, 2, 1).reshape(P_DIM, Q)
    ).astype(bf16)
    S = np.zeros((128, NG * C_OUT), dtype=np.float32)
    for g in range(NG):
        for k in range(128):
            S[k, g * C_OUT + 4 * g + k // C_IN] = 1.0
    S = S.astype(bf16)

    in_maps = []
    for c in range(N_CORES):
        Xc = np.ascontiguousarray(
            X[c * B_SH:(c + 1) * B_SH].reshape(NPOS, C_IN)
        ).astype(bf16)
        XTc = np.ascontiguousarray(Xc.T)
        PTc = np.ascontiguousarray(
            P[c * B_SH:(c + 1) * B_SH].reshape(NPOS, P_DIM).T
        ).astype(bf16)
        in_maps.append({"X": Xc, "XT": XTc, "PT": PTc, "Wk": WkP, "S": S})

    nc = _get_nc()
    trace = os.environ.get("BASS_PROFILE", "0") == "1"
    kw = {}
    if os.environ.get("BASS_TMPDIR"):
        kw["tmpdir"] = os.environ["BASS_TMPDIR"]
    res = run_bass_kernel_spmd(
        nc, in_maps, list(range(N_CORES)), trace=trace, **kw
    )
    LAST_RESULTS = res

    tree_set = set(TREE_CHUNKS)
    out = np.empty((B, N, C_OUT), dtype=np.float32)
    ov = out.reshape(N_CORES, NPOS, C_OUT)
    for c in range(N_CORES):
        o_pos = np.asarray(res.results[c]["out"]).astype(np.float32)
        o_T = np.asarray(res.results[c]["outT"]).astype(np.float32)
        for ch in range(N_CHUNKS):
            lo, hi = ch * CHUNK * TILE_P, (ch + 1) * CHUNK * TILE_P
            if ch in tree_set:
                ov[c, lo:hi] = o_pos[lo:hi]
            else:
                ov[c, lo:hi] = o_T[:, lo:hi].T
    return out


# revision 14
# speedup vs baseline: 1.0522x; 1.0522x over previous
"""Trainium2 Bass kernel for nn_ConditionedDense (hypernetwork-conditioned dense).

Reference computation:
    A = einsum('bnp,pq->bnq', P, Wk)         # hypernetwork: per-position weights
    W = relu(A).reshape(B, N, c_in, c_out)
    out = einsum('bni,bnio->bno', X, W)

Strategy: pure data parallel over 8 NeuronCores (shard batch dim), 16384
positions per core, processed in 16 chunks of 1024 positions. Two pipelines
share the chunks so that PE, ACT and DVE all stay busy:

TREE chunks ([pos, q] layout):
  - PE matmul per 128-pos tile: A = P_tile @ Wk into PSUM
  - ACT relu evicts PSUM -> SBUF bf16
  - DVE multiplies by X (broadcast over o, 2x bf16) and reduces over i with
    a 5-level halving tree of tensor_tensor adds

AT chunks (A^T [q, pos] layout, 2 blocks of 512 positions each):
  - PE matmul per q-chunk g: A^T[q128, pos512] = Wk[:,g]^T-stationary @ P^T
  - eviction+relu+multiply: either fused DVE scalar_tensor_tensor
    ((A max 0) * X_rep, 1x from PSUM) or ACT relu + DVE multiply (2x)
  - the i-reduction runs on the PE: 8 accumulating matmuls with constant
    selection matrices S_g produce out^T[32, 512] in PSUM
  - ACT evicts out^T, host re-transposes

HAM clock-gate management: the PE only un-throttles (1.2 -> 2.4 GHz) under
K=128 matmul activity; K=64 matmuls never warm it and let it re-throttle.
All matmuls here are therefore K=128: P^T lives in persistent [128, ...]
buffers whose bottom 64 partitions are zeroed once (zeros make the padding
exact), and a 12-matmul back-to-back primer warms the PE at kernel start.

Host side (free): P transposed, X transposed layout, Wk column permuted to
q = o*32+i, selection matrices, bf16 casts, output merge.
"""

import os
from contextlib import ExitStack

import numpy as np
import ml_dtypes

import concourse.bass as bass
import concourse.tile as tile
from concourse import bacc, mybir
from concourse.bass_utils import run_bass_kernel_spmd

C_IN = 32
C_OUT = 32
P_DIM = 64
Q = C_IN * C_OUT  # 1024
B, N = 32, 4096
N_CORES = 8
B_SH = B // N_CORES          # 4 batches per core
NPOS = B_SH * N              # 16384 positions per core
TILE_P = 128                 # positions per tile
N_TILES = NPOS // TILE_P     # 128
CHUNK = 8                    # tiles per chunk
N_CHUNKS = N_TILES // CHUNK  # 16
BLK = 512                    # AT-block positions
NG = Q // 128                # 8 q-chunks of 128

N_PRIMER = 18                # redundant K=128 warm-up matmuls
TREE_CHUNKS = (0, 1, 2, 3, 4, 6, 8, 10, 12, 13, 14)
# AT blocks evicted via fused DVE scalar_tensor_tensor: (chunk, block) pairs
STT_BLOCKS = frozenset({(5, 1), (7, 1), (9, 1), (11, 1), (15, 1)})

F32 = mybir.dt.float32
BF16 = mybir.dt.bfloat16

_BUILD_CACHE = {}
LAST_RESULTS = None  # BassKernelResults of the most recent run (for profiling)


def _build_nc():
    nc = bacc.Bacc(
        "TRN2", target_bir_lowering=False, debug=False, num_devices=N_CORES
    )
    X_d = nc.declare_dram_parameter("X", [NPOS, C_IN], BF16, isOutput=False)
    XT_d = nc.declare_dram_parameter("XT", [C_IN, NPOS], BF16, isOutput=False)
    PT_d = nc.declare_dram_parameter("PT", [P_DIM, NPOS], BF16, isOutput=False)
    Wk_d = nc.declare_dram_parameter("Wk", [P_DIM, Q], BF16, isOutput=False)
    S_d = nc.declare_dram_parameter("S", [128, NG * C_OUT], BF16, isOutput=False)
    out_d = nc.declare_dram_parameter("out", [NPOS, C_OUT], BF16, isOutput=True)
    outT_d = nc.declare_dram_parameter("outT", [C_OUT, NPOS], BF16, isOutput=True)

    relu = mybir.ActivationFunctionType.Relu
    mult = mybir.AluOpType.mult
    add = mybir.AluOpType.add
    amax = mybir.AluOpType.max

    tree_set = set(TREE_CHUNKS)

    with ExitStack() as ctx:
        tc = ctx.enter_context(tile.TileContext(nc))
        wkp = ctx.enter_context(tc.tile_pool(name="wk", bufs=1))
        xp = ctx.enter_context(tc.tile_pool(name="x", bufs=2))
        xrp = ctx.enter_context(tc.tile_pool(name="xr", bufs=2))
        pp = ctx.enter_context(tc.tile_pool(name="pT", bufs=1))
        # PSUM: tree pool + AT pool + out2 pool
        tap = ctx.enter_context(tc.tile_pool(name="tpsum", bufs=2, space="PSUM"))
        aap = ctx.enter_context(tc.tile_pool(name="apsum", bufs=3, space="PSUM"))
        o2p = ctx.enter_context(tc.tile_pool(name="o2psum", bufs=1, space="PSUM"))
        hp = ctx.enter_context(tc.tile_pool(name="h", bufs=2))
        hatp = ctx.enter_context(tc.tile_pool(name="hat", bufs=2))
        mp = ctx.enter_context(tc.tile_pool(name="m", bufs=2))
        matp = ctx.enter_context(tc.tile_pool(name="mat", bufs=2))
        t1p = ctx.enter_context(tc.tile_pool(name="t1", bufs=2))
        t2p = ctx.enter_context(tc.tile_pool(name="t2", bufs=2))
        t3p = ctx.enter_context(tc.tile_pool(name="t3", bufs=2))
        t4p = ctx.enter_context(tc.tile_pool(name="t4", bufs=2))
        op = ctx.enter_context(tc.tile_pool(name="o", bufs=3))
        o2sp = ctx.enter_context(tc.tile_pool(name="o2s", bufs=3))

        # S first: its small DMA is the only dependency of the K=128 HAM
        # warm-up primer (emitted in chunk 0), which uses S slices as dummy
        # operands so the PE can warm while the other parameter DMAs run.
        s_t = wkp.tile([128, NG * C_OUT], BF16)
        nc.sync.dma_start(out=s_t[:], in_=S_d[:])

        # Wk padded to K=128: rows 64..127 only ever multiply the zeroed
        # bottom halves of the P^T buffers, so their content is irrelevant;
        # zero them anyway.
        wk_t = wkp.tile([128, Q], BF16)
        nc.gpsimd.memset(wk_t[:], 0)
        nc.sync.dma_start(out=wk_t[0:P_DIM, :], in_=Wk_d[:])

        # persistent P^T ring: [128, 1024] buffers, bottom 64 rows zeroed
        # once so every matmul can run with K=128 (keeps the HAM warm)
        N_PT = 3
        pT_bufs = [
            pp.tile([128, CHUNK * TILE_P], BF16, name=f"pT{i}")
            for i in range(N_PT)
        ]
        for t in pT_bufs:
            nc.gpsimd.memset(t[P_DIM:128, :], 0)

        # deferred emission of an AT block's reduce matmuls + output, so the
        # PE has the next block's gen matmuls queued before a data wait
        pending = []

        def flush_pending():
            while pending:
                fn = pending.pop(0)
                fn()

        def emit_at_reduce(ch, b, m_at):
            def go():
                o2_t = o2p.tile([C_OUT, BLK], F32)
                for g in range(NG):
                    nc.tensor.matmul(
                        o2_t[:], lhsT=s_t[:, bass.ts(g, C_OUT)],
                        rhs=m_at[:, g, :],
                        start=(g == 0), stop=(g == NG - 1),
                    )
                o2s_t = o2sp.tile([C_OUT, BLK], BF16)
                nc.scalar.activation(
                    o2s_t[:], o2_t[:], mybir.ActivationFunctionType.Copy
                )
                nc.sync.dma_start(
                    out=outT_d[:, ch * CHUNK * TILE_P + b * BLK:
                               ch * CHUNK * TILE_P + (b + 1) * BLK],
                    in_=o2s_t[:],
                )
            pending.append(go)

        for ch in range(N_CHUNKS):
            pT_c = pT_bufs[ch % N_PT]
            nc.sync.dma_start(
                out=pT_c[0:P_DIM, :], in_=PT_d[:, bass.ts(ch, TILE_P * CHUNK)]
            )

            if ch in tree_set:
                # ---------------- TREE chunk ----------------
                x_c = xp.tile([TILE_P, CHUNK, C_IN], BF16)
                nc.sync.dma_start(
                    out=x_c[:],
                    in_=X_d[bass.ts(ch, TILE_P * CHUNK), :].rearrange(
                        "(a p) i -> p a i", p=TILE_P
                    ),
                )
                h_c = hp.tile([TILE_P, CHUNK, Q], BF16)
                m_c = mp.tile([TILE_P, CHUNK, Q], BF16)
                o_c = op.tile([TILE_P, CHUNK, C_OUT], BF16)

                for t in range(CHUNK):
                    a_t = tap.tile([TILE_P, Q], F32)
                    if ch == 0 and t == 0:
                        for w in range(N_PRIMER):
                            nc.tensor.matmul(
                                a_t[:, 0:256], lhsT=s_t[:, 0:128],
                                rhs=s_t[:], start=True, stop=True,
                            )
                    lhsT = pT_c[:, bass.ts(t, TILE_P)]
                    nc.tensor.matmul(
                        a_t[:, 0:512], lhsT=lhsT, rhs=wk_t[:, 0:512],
                        start=True, stop=True,
                    )
                    nc.tensor.matmul(
                        a_t[:, 512:1024], lhsT=lhsT, rhs=wk_t[:, 512:1024],
                        start=True, stop=True,
                    )
                    nc.scalar.activation(h_c[:, t, :], a_t[:], relu)
                    if t == 1:
                        flush_pending()

                h4 = h_c[:].rearrange("p j (o i) -> p j o i", o=C_OUT)
                m4 = m_c[:].rearrange("p j (o i) -> p j o i", o=C_OUT)
                x4 = x_c[:].unsqueeze(2).broadcast_to(
                    [TILE_P, CHUNK, C_OUT, C_IN]
                )
                nc.vector.tensor_tensor(out=m4, in0=h4, in1=x4, op=mult)

                t1 = t1p.tile([TILE_P, CHUNK, C_OUT, 16], BF16)
                nc.vector.tensor_tensor(
                    out=t1[:], in0=m4[:, :, :, 0:16], in1=m4[:, :, :, 16:32],
                    op=add,
                )
                t2 = t2p.tile([TILE_P, CHUNK, C_OUT, 8], BF16)
                nc.vector.tensor_tensor(
                    out=t2[:], in0=t1[:, :, :, 0:8], in1=t1[:, :, :, 8:16],
                    op=add,
                )
                t3 = t3p.tile([TILE_P, CHUNK, C_OUT, 4], BF16)
                nc.vector.tensor_tensor(
                    out=t3[:], in0=t2[:, :, :, 0:4], in1=t2[:, :, :, 4:8],
                    op=add,
                )
                t4 = t4p.tile([TILE_P, CHUNK, C_OUT, 2], BF16)
                nc.vector.tensor_tensor(
                    out=t4[:], in0=t3[:, :, :, 0:2], in1=t3[:, :, :, 2:4],
                    op=add,
                )
                nc.vector.tensor_tensor(
                    out=o_c[:].unsqueeze(3), in0=t4[:, :, :, 0:1],
                    in1=t4[:, :, :, 1:2], op=add,
                )
                nc.sync.dma_start(
                    out=out_d[bass.ts(ch, TILE_P * CHUNK), :].rearrange(
                        "(a p) i -> p a i", p=TILE_P
                    ),
                    in_=o_c[:],
                )
            else:
                # ---------------- AT chunk (2 blocks of 512 pos) ----------
                xr_c = xrp.tile([128, 2, BLK], BF16)
                for r in range(4):
                    nc.sync.dma_start(
                        out=xr_c[32 * r:32 * (r + 1), :, :],
                        in_=XT_d[:, bass.ts(ch, TILE_P * CHUNK)].rearrange(
                            "i (b c) -> i b c", b=2
                        ),
                    )
                for b in range(2):
                    is_stt = (ch, b) in STT_BLOCKS
                    m_at = matp.tile([128, NG, BLK], BF16)
                    h_at = None if is_stt else hatp.tile([128, NG, BLK], BF16)
                    for g in range(NG):
                        a_at = aap.tile([128, BLK], F32)
                        nc.tensor.matmul(
                            a_at[:], lhsT=wk_t[:, bass.ts(g, 128)],
                            rhs=pT_c[:, bass.ts(b, BLK)],
                            start=True, stop=True,
                        )
                        if is_stt:
                            nc.vector.scalar_tensor_tensor(
                                out=m_at[:, g, :], in0=a_at[:], scalar=0.0,
                                in1=xr_c[:, b, :], op0=amax, op1=mult,
                            )
                        else:
                            nc.scalar.activation(h_at[:, g, :], a_at[:], relu)
                        if g == 1:
                            flush_pending()
                    if not is_stt:
                        nc.vector.tensor_tensor(
                            out=m_at[:], in0=h_at[:],
                            in1=xr_c[:, b, :].unsqueeze(1).broadcast_to(
                                [128, NG, BLK]
                            ),
                            op=mult,
                        )
                    emit_at_reduce(ch, b, m_at)
        flush_pending()

    nc.finalize()
    return nc


def _get_nc():
    key = "v5"
    if key not in _BUILD_CACHE:
        _BUILD_CACHE[key] = _build_nc()
    return _BUILD_CACHE[key]


def kernel(X, P, Wk):
    global LAST_RESULTS
    X = np.asarray(X, dtype=np.float32)
    P = np.asarray(P, dtype=np.float32)
    Wk = np.asarray(Wk, dtype=np.float32)
    bf16 = ml_dtypes.bfloat16

    # Host-side prep (free): shard, transpose P/X, permute Wk columns so the
    # device-side layout is q = o*32 + i; selection matrices for the PE
    # reduction; cast matmul operands to bf16.
    WkP = np.ascontiguousarray(
        Wk.reshape(P_DIM, C_IN, C_OUT).transpose(0, 2, 1).reshape(P_DIM, Q)
    ).astype(bf16)
    S = np.zeros((128, NG * C_OUT), dtype=np.float32)
    for g in range(NG):
        for k in range(128):
            S[k, g * C_OUT + 4 * g + k // C_IN] = 1.0
    S = S.astype(bf16)

    in_maps = []
    for c in range(N_CORES):
        Xc = np.ascontiguousarray(
            X[c * B_SH:(c + 1) * B_SH].reshape(NPOS, C_IN)
        ).astype(bf16)
        XTc = np.ascontiguousarray(Xc.T)
        PTc = np.ascontiguousarray(
            P[c * B_SH:(c + 1) * B_SH].reshape(NPOS, P_DIM).T
        ).astype(bf16)
        in_maps.append({"X": Xc, "XT": XTc, "PT": PTc, "Wk": WkP, "S": S})

    nc = _get_nc()
    trace = os.environ.get("BASS_PROFILE", "0") == "1"
    kw = {}
    if os.environ.get("BASS_TMPDIR"):
        kw["tmpdir"] = os.environ["BASS_TMPDIR"]
    res = run_bass_kernel_spmd(
        nc, in_maps, list(range(N_CORES)), trace=trace, **kw
    )
    LAST_RESULTS = res

    tree_set = set(TREE_CHUNKS)
    out = np.empty((B, N, C_OUT), dtype=np.float32)
    ov = out.reshape(N_CORES, NPOS, C_OUT)
    for c in range(N_CORES):
        o_pos = np.asarray(res.results[c]["out"]).astype(np.float32)
        o_T = np.asarray(res.results[c]["outT"]).astype(np.float32)
        for ch in range(N_CHUNKS):
            lo, hi = ch * CHUNK * TILE_P, (ch + 1) * CHUNK * TILE_P
            if ch in tree_set:
                ov[c, lo:hi] = o_pos[lo:hi]
            else:
                ov[c, lo:hi] = o_T[:, lo:hi].T
    return out
